# revision 7
# baseline (speedup 1.0000x reference)
"""Trainium2 Bass kernel for nn_MultiHeadAttention (B=2, S=2048, H=1024, 16 heads).

Sharding: 8 cores = 2 (batch) x 4 (head-groups of 4 heads). Each core computes
QKV projections for its 256-dim head slice, attention for its 4 heads, and a
partial output projection. Host sums the 4 head-group partials per batch and
adds the output bias.

On-chip layout: activations live transposed as [d, s] with the hidden/head dim
on partitions, so every matmul contraction runs on the PE partition axis.
Attention uses unnormalized exp scores with a fused ones-column in V to
produce row sums, normalizing the small [64, S] per-head output.

Schedule notes: steady state is gated by the scalar engine's exp chain
(16 x 1.15us exps per head-pair) with the PE slightly under-loaded, so PE
micro-idles trip the HAM governor and halve the PE clock. This version
(1) starts the exp chain ~25us earlier by interleaving the K projection
ns-chunks with the first pair's score groups (input DMA is issued in
512-column slices in consumption order), (2) spreads every dependency-free
matmul (output projection in 2-matmul chunks, next-block Q projection, V
projection / V' transposes) into the slots where scores stall on the exp
PSUM rotation, and (3) merges each pair's two per-head accumulators into one
two-bank PSUM tile so the softmax-sum reciprocal chain is one Ln + one Exp
per pair instead of four scalar ops.
"""

import sys

if "/opt/trn_rl_repo" not in sys.path:
    sys.path.insert(0, "/opt/trn_rl_repo")

import numpy as np

HIDDEN, HEADS, D_K, B, S = 1024, 16, 64, 2, 2048
G = 4              # head groups (tensor-parallel dim)
HPG = HEADS // G   # heads per group
DSL = HPG * D_K    # 256: d-slice per core
P = 128
QB = 512           # q-block size for attention tiling
N_QB = S // QB     # 4
KC = S // P        # 16 k-chunks
NG = KC // 2       # 8 two-chunk groups
CC = HIDDEN // P   # 8 contraction chunks for projections
NS = S // 512      # 4 ns (key/seq 512-col) chunks
SCALE = 1.0 / np.sqrt(np.float32(D_K))


def _build_nc():
    from contextlib import ExitStack

    import concourse.mybir as mybir
    import concourse.tile as tile
    from concourse.bacc import Bacc
    from concourse.masks import make_identity

    dt = mybir.dt
    f32 = dt.float32
    f16 = dt.float16

    nc = Bacc(None)

    qT_d = nc.dram_tensor("qT", [HIDDEN, S], f16, kind="ExternalInput")
    kT_d = nc.dram_tensor("kT", [HIDDEN, S], f16, kind="ExternalInput")
    vT_d = nc.dram_tensor("vT", [HIDDEN, S], f16, kind="ExternalInput")
    wqT_d = nc.dram_tensor("wqT", [HIDDEN, DSL], f16, kind="ExternalInput")
    wkT_d = nc.dram_tensor("wkT", [HIDDEN, DSL], f16, kind="ExternalInput")
    wvT_d = nc.dram_tensor("wvT", [HIDDEN, DSL], f16, kind="ExternalInput")
    woT_d = nc.dram_tensor("woT", [DSL, HIDDEN], f16, kind="ExternalInput")
    bq_d = nc.dram_tensor("bq", [DSL], f32, kind="ExternalInput")
    bk_d = nc.dram_tensor("bk", [DSL], f32, kind="ExternalInput")
    bv_d = nc.dram_tensor("bv", [DSL], f32, kind="ExternalInput")
    y_d = nc.dram_tensor("y", [S, HIDDEN], f16, kind="ExternalOutput")
    y_r = y_d.rearrange("(sc p) e -> p sc e", p=P)

    w_r = {
        "q": wqT_d.rearrange("(c p) d -> p c d", p=P),
        "k": wkT_d.rearrange("(c p) d -> p c d", p=P),
        "v": wvT_d.rearrange("(c p) d -> p c d", p=P),
    }
    b_r = {"q": bq_d, "k": bk_d, "v": bv_d}
    x_r = {
        "q": qT_d.rearrange("(c p) s -> p c s", p=P),
        "k": kT_d.rearrange("(c p) s -> p c s", p=P),
        "v": vT_d.rearrange("(c p) s -> p c s", p=P),
    }

    with tile.TileContext(nc) as tc:
        with (
            tc.tile_pool(name="weights", bufs=1) as wpool,
            tc.tile_pool(name="qkvT", bufs=1) as qkvT_pool,
            tc.tile_pool(name="xT_out", bufs=1) as xT_pool,
            tc.tile_pool(name="small", bufs=1) as small,
            tc.tile_pool(name="expT", bufs=2) as exp_pool,
            tc.tile_pool(name="norm", bufs=2) as norm_pool,
            tc.tile_pool(name="y_out", bufs=1) as ypool,
            tc.tile_pool(name="xq", bufs=2) as xqpool,
            tc.tile_pool(name="sc_ps", bufs=2, space="PSUM") as sc_ps,
            tc.tile_pool(name="acc_ps", bufs=1, space="PSUM") as acc_ps,
            tc.tile_pool(name="rby_ps", bufs=2, space="PSUM") as rby_ps,
        ):
            nc.scalar.add_instruction(
                mybir.InstLoadActFuncSet(
                    name=nc.get_next_instruction_name(),
                    ins=[],
                    outs=[],
                    act_func_set_id=6,  # natural_log_exp_and_others
                )
            )
            ident = small.tile([P, P], f16)
            make_identity(nc, ident)
            ones16 = small.tile([P, D_K], f16, tag="ones")
            nc.vector.memset(ones16[:], 1.0)

            proj_out = {}
            for name in ("k", "v", "q"):
                proj_out[name] = qkvT_pool.tile(
                    [P, DSL // P, S], f16, tag=f"{name}T", name=f"{name}T"
                )
            QT, KT, VT = proj_out["q"], proj_out["k"], proj_out["v"]

            w_sb = {}
            b_sb = {}

            def issue_wb(name):
                w_t = wpool.tile([P, CC, DSL], f16, tag=f"w{name}")
                for wh in range(2):
                    nc.sync.dma_start(
                        w_t[:, :, wh * P : (wh + 1) * P],
                        w_r[name][:, :, wh * P : (wh + 1) * P],
                    )
                b_t = small.tile([P, DSL // P], f32, tag=f"b{name}")
                nc.sync.dma_start(b_t[:], b_r[name].rearrange("(o p) -> p o", p=P))
                w_sb[name], b_sb[name] = w_t, b_t

            es_k = ExitStack()
            xkpool = es_k.enter_context(tc.tile_pool(name="xk", bufs=1))
            es_v = ExitStack()
            xq_tiles = {}
            xk_tiles = []
            xv_tiles = []

            def issue_xq(qb_):
                for qt in range(4):
                    t = xqpool.tile([P, 2, QB], f16, tag=f"xq{qt}",
                                    name=f"xq{qb_}{qt}")
                    nc.sync.dma_start(
                        t[:],
                        x_r["q"][:, 2 * qt : 2 * qt + 2,
                                 qb_ * QB : (qb_ + 1) * QB],
                    )
                    xq_tiles[(qt, qb_)] = t

            def issue_xkv_ns(name, tiles, ns):
                # one 512-column slice of the k or v input, all 4 cc-pair tiles
                for qt in range(4):
                    nc.sync.dma_start(
                        tiles[qt][:, :, ns * 512 : (ns + 1) * 512],
                        x_r[name][:, 2 * qt : 2 * qt + 2,
                                  ns * 512 : (ns + 1) * 512],
                    )

            # ---- input DMA, in consumption order ----
            issue_wb("k")
            for qt in range(4):
                xk_tiles.append(xkpool.tile([P, 2, S], f16, tag=f"xk{qt}", name=f"xk{qt}"))
            issue_xkv_ns("k", xk_tiles, 0)
            issue_wb("q")
            issue_xq(0)
            issue_xkv_ns("k", xk_tiles, 1)
            issue_xkv_ns("k", xk_tiles, 2)
            issue_xkv_ns("k", xk_tiles, 3)
            issue_xq(1)
            issue_wb("v")

            # ---- emission helpers (all PSUM via the shared pools) ----
            def emit_proj(name, dst, ns, mc, get_src):
                ps = rby_ps.tile([P, 512], f32, tag="rby",
                                 name=f"{name}p{ns}{mc}")
                for cc in range(CC):
                    nc.tensor.matmul(
                        ps[:],
                        w_sb[name][:, cc, mc * P : (mc + 1) * P],
                        get_src(cc),
                        start=(cc == 0),
                        stop=(cc == CC - 1),
                    )
                nc.vector.tensor_scalar_add(
                    dst[:, mc, ns * 512 : (ns + 1) * 512],
                    ps[:],
                    b_sb[name][:, mc : mc + 1],
                )

            def emit_kproj(ns, mc):
                emit_proj("k", KT, ns, mc,
                          lambda cc: xk_tiles[cc // 2][
                              :, cc % 2, ns * 512 : (ns + 1) * 512])

            def emit_vproj(mc, ns):
                emit_proj("v", VT, ns, mc,
                          lambda cc: xv_tiles[cc // 2][
                              :, cc % 2, ns * 512 : (ns + 1) * 512])

            def emit_qproj(qb_, mc):
                emit_proj("q", QT, qb_, mc,
                          lambda cc: xq_tiles[(cc // 2, qb_)][:, cc % 2, :])

            vprime = [None] * HPG
            XT = xT_pool.tile([P, DSL // P, S], f16, tag="XT")

            def emit_vprime(hs):
                # V' build: PE-transpose VT 64x128 blocks into [s, d] tiles
                # with a ones column (row-sum trick).
                for h in hs:
                    vp = xT_pool.tile([P, KC, D_K + 1], f16, tag=f"vp{h}")
                    nc.vector.memset(vp[:], 1.0)
                    hc, hp = divmod(h, 2)
                    pb = hp * D_K
                    idn = ident[pb : pb + D_K, pb : pb + D_K]
                    for kc4 in range(KC // 4):
                        tp = rby_ps.tile([P, 4, D_K], f16, tag="rby",
                                         name=f"vt{h}{kc4}")
                        for j in range(4):
                            kc = kc4 * 4 + j
                            nc.tensor.transpose(
                                tp[:, j, :],
                                VT[pb : pb + D_K, hc,
                                   kc * P : (kc + 1) * P],
                                idn,
                            )
                        nc.vector.tensor_copy(
                            vp[:, kc4 * 4 : kc4 * 4 + 4, 0:D_K], tp[:]
                        )
                    vprime[h] = vp

            def emit_norm_late(ctx):
                # broadcast the reciprocal row across partitions with a K=1
                # fp16 matmul, then scale the unnormalized head outputs.
                for hc, qb_, xun, rec16 in ctx:
                    qs_ = slice(qb_ * QB, (qb_ + 1) * QB)
                    for hp in range(2):
                        rb_ps = rby_ps.tile([D_K, QB], f32, tag="rby",
                                            name=f"rb{hc}{hp}")
                        nc.tensor.matmul(
                            rb_ps[:],
                            ones16[0:1, :],
                            rec16[0:1, hp, :],
                            start=True,
                            stop=True,
                        )
                        if hp == 0:
                            nc.vector.tensor_tensor(
                                XT[0:D_K, hc, qs_], xun[:, 0, :], rb_ps[:],
                                mybir.AluOpType.mult,
                            )
                        else:
                            tmp = norm_pool.tile([D_K, QB], f16, tag="xtmp")
                            nc.vector.tensor_tensor(
                                tmp[:], xun[:, 1, :], rb_ps[:],
                                mybir.AluOpType.mult,
                            )
                            nc.sync.dma_start(XT[D_K:P, hc, qs_], tmp[:])

            def emit_outproj_chunk(qb_, sc4, ec, y_sb):
                # one ec-half of one 128-seq chunk: 2 matmuls + 1 copy
                sc = qb_ * 4 + sc4
                ps = rby_ps.tile([P, 512], f32, tag="rby",
                                 name=f"yp{sc4}{ec}")
                for dc in range(DSL // P):
                    nc.tensor.matmul(
                        ps[:],
                        XT[:, dc, sc * P : (sc + 1) * P],
                        woT_sb[:, dc, ec * 512 : (ec + 1) * 512],
                        start=(dc == 0),
                        stop=(dc == DSL // P - 1),
                    )
                nc.vector.tensor_copy(
                    y_sb[:, sc4, ec * 512 : (ec + 1) * 512], ps[:]
                )

            def emit_epilogue(hc, qb_, acc):
                # drain the pair's acc right away: one copy of both heads'
                # unnormalized outputs, one Ln + one Exp for both row-sum
                # reciprocals (1/sum = exp(-ln(sum)); the
                # natural_log_exp_and_others table serves both).
                xun = norm_pool.tile([D_K, 2, QB], f32, tag="xun",
                                     name=f"xun{hc}")
                nc.vector.tensor_copy(xun[:], acc[0:D_K, :, :])
                lnr = norm_pool.tile([1, 2, QB], f32, tag="lnr",
                                     name=f"ln{hc}", bufs=1)
                nc.scalar.activation(
                    lnr[:],
                    acc[D_K : D_K + 1, :, :],
                    mybir.ActivationFunctionType.Ln,
                )
                rec16 = norm_pool.tile([1, 2, QB], f16, tag="rec16",
                                       name=f"rc{hc}")
                nc.scalar.activation(
                    rec16[:],
                    lnr[:],
                    mybir.ActivationFunctionType.Exp,
                    scale=-1.0,
                )
                return [(hc, qb_, xun, rec16)]

            def emit_scores(heads, qb_, g, expm):
                # j-outer: the two heads' same-kc matmuls are adjacent and
                # live in disjoint PE row groups (bases 0/64).
                qs = slice(qb_ * QB, (qb_ + 1) * QB)
                hc = heads[0] // 2
                tiles = {}
                for h in heads:
                    tiles[h] = sc_ps.tile([P, 2, QB], f32, tag="sc",
                                          name=f"sc{h}{g}")
                for j in range(2):
                    kc = 2 * g + j
                    for h in heads:
                        hp = h & 1
                        pb = hp * D_K
                        nc.tensor.matmul(
                            tiles[h][:, j, :],
                            KT[pb : pb + D_K, hc, kc * P : (kc + 1) * P],
                            QT[pb : pb + D_K, hc, qs],
                            start=True,
                            stop=True,
                            tile_position=(pb, 0),
                        )
                for h in heads:
                    hp = h & 1
                    nc.scalar.activation(
                        expm[:, 2 * g : 2 * g + 2, hp, :],
                        tiles[h][:],
                        mybir.ActivationFunctionType.Exp,
                        scale=float(SCALE),
                    )

            def emit_attnv(heads, acc, expm, kcs):
                for kc in kcs:
                    for h in heads:
                        hp = h & 1
                        nc.tensor.matmul(
                            acc[0 : D_K + 1, hp, :],
                            vprime[h][:, kc, :],
                            expm[:, kc, hp, :],
                            start=(kc == 0),
                            stop=(kc == KC - 1),
                        )

            # ---- startup: K proj ns0 + Q proj qb0, then pair 0 with
            #      interleaved K proj / V proj fillers ----
            emit_kproj(0, 0)
            emit_kproj(0, 1)
            emit_qproj(0, 0)
            emit_qproj(0, 1)

            heads0 = (0, 1)
            heads1 = (2, 3)
            expm0 = exp_pool.tile([P, KC, 2, QB], f16, tag="exp", name="ex00")
            for g in range(NG):
                if g <= 5:
                    emit_kproj(1 + g // 2, g % 2)
                else:
                    emit_qproj(1, g - 6)
                emit_scores(heads0, 0, g, expm0)
            es_k.close()

            # xv staged only now (xk's 32KB freed); V DMA follows the k/q data
            xvpool = es_v.enter_context(tc.tile_pool(name="xv", bufs=1))
            for qt in range(4):
                xv_tiles.append(xvpool.tile([P, 2, S], f16, tag=f"xv{qt}",
                                            name=f"xv{qt}"))
            for ns in range(4):
                issue_xkv_ns("v", xv_tiles, ns)
            woT_sb = wpool.tile([P, DSL // P, HIDDEN], f16, tag="wo")
            nc.sync.dma_start(woT_sb[:], woT_d.rearrange("(c p) e -> p c e", p=P))

            # ---- qb0 pair 1: all remaining V work + pair-0 attn@V ----
            expm1 = exp_pool.tile([P, KC, 2, QB], f16, tag="exp", name="ex01")
            acc0 = acc_ps.tile([P, 2, QB], f32, tag="acc", name="acc00")
            pending_norm = None
            D1 = 6
            for g in range(NG + D1):
                if g <= 3:
                    emit_vproj(0, g)
                elif g == 4:
                    emit_vprime(heads0)
                elif g == 5:
                    emit_attnv(heads0, acc0, expm0, range(0, 4))
                    emit_vproj(1, 0)
                elif g == 6:
                    emit_attnv(heads0, acc0, expm0, range(4, 8))
                    emit_vproj(1, 1)
                elif g == 7:
                    emit_attnv(heads0, acc0, expm0, range(8, 12))
                    emit_vproj(1, 2)
                elif g == 8:
                    emit_attnv(heads0, acc0, expm0, range(12, 16))
                    emit_vproj(1, 3)
                    pending_norm = emit_epilogue(0, 0, acc0)
                elif g == 9:
                    emit_vprime(heads1)
                elif g == 10:
                    acc1 = acc_ps.tile([P, 2, QB], f32, tag="acc",
                                       name="acc01")
                    emit_attnv(heads1, acc1, expm1, range(0, 4))
                    issue_xq(2)
                    emit_norm_late(pending_norm)
                    pending_norm = None
                elif g == 11:
                    emit_attnv(heads1, acc1, expm1, range(4, 8))
                    es_v.close()
                elif g == 12:
                    emit_attnv(heads1, acc1, expm1, range(8, 12))
                elif g == 13:
                    emit_attnv(heads1, acc1, expm1, range(12, 16))
                if g < NG:
                    emit_scores(heads1, 0, g, expm1)
            pending_norm = emit_epilogue(1, 0, acc1)
            pending_outproj = 0

            # ---- steady state: q-blocks 1..3, D=2 ----
            # boundary-first filler placement: the in-order PE queue stalls at
            # scores(P+1, g0) until the previous pair's exps drain, so the
            # dependency-free outproj / qproj matmuls go at pair ENDS (g8/g9
            # and after the epilogue), bridging the exp backlog.
            D2 = 2
            for qb in range(1, N_QB):
                for hpair in range(2):
                    heads = (2 * hpair, 2 * hpair + 1)
                    expm = exp_pool.tile([P, KC, 2, QB], f16, tag="exp",
                                         name=f"ex{qb}{hpair}")
                    acc = None
                    if hpair == 0:
                        y_sb = ypool.tile([P, 4, HIDDEN], f16, tag="y",
                                          name=f"y{qb}")
                    c0 = 4 * hpair  # this pair's 4 outproj chunks
                    for g in range(NG + D2):
                        if g == D2:
                            acc = acc_ps.tile([P, 2, QB], f32, tag="acc",
                                              name=f"acc{qb}{hpair}")
                        if g >= D2:
                            emit_attnv(heads, acc, expm,
                                       (2 * (g - D2), 2 * (g - D2) + 1))
                        if g == 2 and pending_norm is not None:
                            emit_norm_late(pending_norm)
                            pending_norm = None
                        if g in (8, 9) and pending_outproj is not None:
                            c = c0 + (g - 8)
                            emit_outproj_chunk(pending_outproj, c // 2,
                                               c % 2, y_sb)
                        if g < NG:
                            emit_scores(heads, qb, g, expm)
                    pending_norm = emit_epilogue(hpair, qb, acc)
                    # post-epilogue boundary bridge
                    if pending_outproj is not None:
                        for c in (c0 + 2, c0 + 3):
                            emit_outproj_chunk(pending_outproj, c // 2,
                                               c % 2, y_sb)
                        if hpair == 1:
                            nc.sync.dma_start(
                                y_r[:, pending_outproj * 4 :
                                    pending_outproj * 4 + 4, :],
                                y_sb[:],
                            )
                    if qb < N_QB - 1:
                        emit_qproj(qb + 1, hpair)
                        if hpair == 1 and qb + 2 < N_QB:
                            issue_xq(qb + 2)
                pending_outproj = qb

            # tail
            emit_norm_late(pending_norm)
            y_sb = ypool.tile([P, 4, HIDDEN], f16, tag="y", name="ytail")
            for c in range(8):
                emit_outproj_chunk(pending_outproj, c // 2, c % 2, y_sb)
                if c % 2 == 1:
                    nc.sync.dma_start(
                        y_r[:, pending_outproj * 4 + c // 2 :
                            pending_outproj * 4 + c // 2 + 1, :],
                        y_sb[:, c // 2 : c // 2 + 1, :],
                    )

    nc.finalize()
    return nc


_NC_CACHE = None


def _get_nc():
    global _NC_CACHE
    if _NC_CACHE is None:
        _NC_CACHE = _build_nc()
    return _NC_CACHE


def make_in_maps(q, k, v, Wq, bq, Wk, bk, Wv, bv, Wo):
    """Host-side sharding: per-core input dicts (core = b * G + g)."""
    f16 = np.float16
    qT = [np.ascontiguousarray(q[b].T).astype(f16) for b in range(B)]
    kT = [np.ascontiguousarray(k[b].T).astype(f16) for b in range(B)]
    vT = [np.ascontiguousarray(v[b].T).astype(f16) for b in range(B)]
    in_maps = []
    for core in range(B * G):
        b, g = divmod(core, G)
        sl = slice(g * DSL, (g + 1) * DSL)
        in_maps.append(
            {
                "qT": qT[b],
                "kT": kT[b],
                "vT": vT[b],
                "wqT": np.ascontiguousarray(Wq[sl, :].T).astype(f16),
                "wkT": np.ascontiguousarray(Wk[sl, :].T).astype(f16),
                "wvT": np.ascontiguousarray(Wv[sl, :].T).astype(f16),
                "woT": np.ascontiguousarray(Wo[:, sl].T).astype(f16),
                "bq": np.ascontiguousarray(bq[sl], np.float32),
                "bk": np.ascontiguousarray(bk[sl], np.float32),
                "bv": np.ascontiguousarray(bv[sl], np.float32),
            }
        )
    return in_maps


def kernel(q, k, v, Wq, bq, Wk, bk, Wv, bv, Wo, bo):
    from concourse.bass_utils import run_bass_kernel_spmd

    q, k, v = (np.asarray(a, np.float32) for a in (q, k, v))
    Wq, Wk, Wv, Wo = (np.asarray(a, np.float32) for a in (Wq, Wk, Wv, Wo))
    bq, bk, bv, bo = (np.asarray(a, np.float32) for a in (bq, bk, bv, bo))

    nc = _get_nc()
    in_maps = make_in_maps(q, k, v, Wq, bq, Wk, bk, Wv, bv, Wo)
    res = run_bass_kernel_spmd(nc, in_maps, core_ids=list(range(B * G)))

    out = np.zeros((B, S, HIDDEN), np.float32)
    for b in range(B):
        acc = np.zeros((S, HIDDEN), np.float32)
        for g in range(G):
            acc += res.results[b * G + g]["y"].astype(np.float32)
        out[b] = acc + bo
    return out


# revision 12
# speedup vs baseline: 1.2144x; 1.2144x over previous
"""Trainium2 Bass kernel for nn_MultiHeadAttention (B=2, S=2048, H=1024, 16 heads).

Sharding: 8 cores = 2 (batch) x 4 (head-groups of 4 heads). Each core computes
QKV projections for its 256-dim head slice, attention for its 4 heads, and a
partial output projection. Host sums the 4 head-group partials per batch and
adds the output bias.

On-chip layout: activations live transposed as [d, s] with the hidden/head dim
on partitions, so every matmul contraction runs on the PE partition axis.
Attention uses unnormalized exp scores with a fused ones-column in V to
produce row sums, normalizing the small [64, S] per-head output.

Schedule notes: steady state is gated by the scalar engine's exp chain
(16 x 1.15us exps per head-pair) with the PE slightly under-loaded, so PE
micro-idles trip the HAM governor and halve the PE clock. This version
(1) starts the exp chain ~25us earlier by interleaving the K projection
ns-chunks with the first pair's score groups (input DMA is issued in
512-column slices in consumption order), (2) spreads every dependency-free
matmul (output projection in 2-matmul chunks, next-block Q projection, V
projection / V' transposes) into the slots where scores stall on the exp
PSUM rotation, and (3) merges each pair's two per-head accumulators into one
two-bank PSUM tile so the softmax-sum reciprocal chain is one Ln + one Exp
per pair instead of four scalar ops.
"""

import sys

if "/opt/trn_rl_repo" not in sys.path:
    sys.path.insert(0, "/opt/trn_rl_repo")

import numpy as np

HIDDEN, HEADS, D_K, B, S = 1024, 16, 64, 2, 2048
G = 4              # head groups (tensor-parallel dim)
HPG = HEADS // G   # heads per group
DSL = HPG * D_K    # 256: d-slice per core
P = 128
QB = 512           # q-block size for attention tiling
N_QB = S // QB     # 4
KC = S // P        # 16 k-chunks
NG = KC // 2       # 8 two-chunk groups
CC = HIDDEN // P   # 8 contraction chunks for projections
NS = S // 512      # 4 ns (key/seq 512-col) chunks
SCALE = 1.0 / np.sqrt(np.float32(D_K))


def _build_nc():
    from contextlib import ExitStack

    import concourse.mybir as mybir
    import concourse.tile as tile
    from concourse.bacc import Bacc
    from concourse.masks import make_identity

    dt = mybir.dt
    f32 = dt.float32
    f16 = dt.float16

    nc = Bacc(None)

    qT_d = nc.dram_tensor("qT", [HIDDEN, S], f16, kind="ExternalInput")
    kT_d = nc.dram_tensor("kT", [HIDDEN, S], f16, kind="ExternalInput")
    vT_d = nc.dram_tensor("vT", [HIDDEN, S], f16, kind="ExternalInput")
    wqT_d = nc.dram_tensor("wqT", [HIDDEN, DSL], f16, kind="ExternalInput")
    wkT_d = nc.dram_tensor("wkT", [HIDDEN, DSL], f16, kind="ExternalInput")
    wvT_d = nc.dram_tensor("wvT", [HIDDEN, DSL], f16, kind="ExternalInput")
    woT_d = nc.dram_tensor("woT", [DSL, HIDDEN], f16, kind="ExternalInput")
    bq_d = nc.dram_tensor("bq", [DSL], f32, kind="ExternalInput")
    bk_d = nc.dram_tensor("bk", [DSL], f32, kind="ExternalInput")
    bv_d = nc.dram_tensor("bv", [DSL], f32, kind="ExternalInput")
    y_d = nc.dram_tensor("y", [S, HIDDEN], f16, kind="ExternalOutput")
    y_r = y_d.rearrange("(sc p) e -> p sc e", p=P)

    w_r = {
        "q": wqT_d.rearrange("(c p) d -> p c d", p=P),
        "k": wkT_d.rearrange("(c p) d -> p c d", p=P),
        "v": wvT_d.rearrange("(c p) d -> p c d", p=P),
    }
    b_r = {"q": bq_d, "k": bk_d, "v": bv_d}
    x_r = {
        "q": qT_d.rearrange("(c p) s -> p c s", p=P),
        "k": kT_d.rearrange("(c p) s -> p c s", p=P),
        "v": vT_d.rearrange("(c p) s -> p c s", p=P),
    }

    with tile.TileContext(nc) as tc:
        with (
            tc.tile_pool(name="weights", bufs=1) as wpool,
            tc.tile_pool(name="qkvT", bufs=1) as qkvT_pool,
            tc.tile_pool(name="xT_out", bufs=1) as xT_pool,
            tc.tile_pool(name="small", bufs=1) as small,
            tc.tile_pool(name="expT", bufs=2) as exp_pool,
            tc.tile_pool(name="norm", bufs=2) as norm_pool,
            tc.tile_pool(name="y_out", bufs=1) as ypool,
            tc.tile_pool(name="xq", bufs=2) as xqpool,
            tc.tile_pool(name="sc_ps", bufs=2, space="PSUM") as sc_ps,
            tc.tile_pool(name="acc_ps", bufs=1, space="PSUM") as acc_ps,
            tc.tile_pool(name="rby_ps", bufs=2, space="PSUM") as rby_ps,
        ):
            nc.scalar.add_instruction(
                mybir.InstLoadActFuncSet(
                    name=nc.get_next_instruction_name(),
                    ins=[],
                    outs=[],
                    act_func_set_id=6,  # natural_log_exp_and_others
                )
            )
            ident = small.tile([P, P], f16)
            make_identity(nc, ident)
            ones16 = small.tile([P, D_K], f16, tag="ones")
            nc.vector.memset(ones16[:], 1.0)

            proj_out = {}
            for name in ("k", "v", "q"):
                proj_out[name] = qkvT_pool.tile(
                    [P, DSL // P, S], f16, tag=f"{name}T", name=f"{name}T"
                )
            QT, KT, VT = proj_out["q"], proj_out["k"], proj_out["v"]

            w_sb = {}
            b_sb = {}

            def issue_wb(name):
                w_t = wpool.tile([P, CC, DSL], f16, tag=f"w{name}")
                for wh in range(2):
                    nc.sync.dma_start(
                        w_t[:, :, wh * P : (wh + 1) * P],
                        w_r[name][:, :, wh * P : (wh + 1) * P],
                    )
                b_t = small.tile([P, DSL // P], f32, tag=f"b{name}")
                nc.sync.dma_start(b_t[:], b_r[name].rearrange("(o p) -> p o", p=P))
                w_sb[name], b_sb[name] = w_t, b_t

            es_k = ExitStack()
            xkpool = es_k.enter_context(tc.tile_pool(name="xk", bufs=1))
            es_v = ExitStack()
            xq_tiles = {}
            xk_tiles = []
            xv_tiles = []

            def issue_xq(qb_):
                for qt in range(4):
                    t = xqpool.tile([P, 2, QB], f16, tag=f"xq{qt}",
                                    name=f"xq{qb_}{qt}")
                    nc.sync.dma_start(
                        t[:],
                        x_r["q"][:, 2 * qt : 2 * qt + 2,
                                 qb_ * QB : (qb_ + 1) * QB],
                    )
                    xq_tiles[(qt, qb_)] = t

            def issue_xkv_ns(name, tiles, ns):
                # one 512-column slice of the k or v input, all 4 cc-pair tiles
                for qt in range(4):
                    nc.sync.dma_start(
                        tiles[qt][:, :, ns * 512 : (ns + 1) * 512],
                        x_r[name][:, 2 * qt : 2 * qt + 2,
                                  ns * 512 : (ns + 1) * 512],
                    )

            # ---- input DMA, in consumption order ----
            issue_wb("k")
            for qt in range(4):
                xk_tiles.append(xkpool.tile([P, 2, S], f16, tag=f"xk{qt}", name=f"xk{qt}"))
            issue_xkv_ns("k", xk_tiles, 0)
            issue_wb("q")
            issue_xq(0)
            issue_wb("v")
            issue_xkv_ns("k", xk_tiles, 1)
            issue_xkv_ns("k", xk_tiles, 2)
            issue_xkv_ns("k", xk_tiles, 3)
            issue_xq(1)

            # ---- emission helpers (all PSUM via the shared pools) ----
            def emit_proj(name, dst, ns, mc, get_src):
                ps = rby_ps.tile([P, 512], f32, tag="rby",
                                 name=f"{name}p{ns}{mc}")
                for cc in range(CC):
                    nc.tensor.matmul(
                        ps[:],
                        w_sb[name][:, cc, mc * P : (mc + 1) * P],
                        get_src(cc),
                        start=(cc == 0),
                        stop=(cc == CC - 1),
                    )
                nc.vector.tensor_scalar_add(
                    dst[:, mc, ns * 512 : (ns + 1) * 512],
                    ps[:],
                    b_sb[name][:, mc : mc + 1],
                )

            def emit_kproj(ns, mc):
                emit_proj("k", KT, ns, mc,
                          lambda cc: xk_tiles[cc // 2][
                              :, cc % 2, ns * 512 : (ns + 1) * 512])

            def emit_vproj(mc, ns):
                emit_proj("v", VT, ns, mc,
                          lambda cc: xv_tiles[cc // 2][
                              :, cc % 2, ns * 512 : (ns + 1) * 512])

            def emit_qproj(qb_, mc):
                emit_proj("q", QT, qb_, mc,
                          lambda cc: xq_tiles[(cc // 2, qb_)][:, cc % 2, :])

            vprime = [None] * HPG
            XT = xT_pool.tile([P, DSL // P, S], f16, tag="XT")

            def emit_vprime(hs):
                # V' build: PE-transpose VT 64x128 blocks into [s, d] tiles
                # with a ones column (row-sum trick).
                for h in hs:
                    vp = xT_pool.tile([P, KC, D_K + 1], f16, tag=f"vp{h}")
                    nc.vector.memset(vp[:], 1.0)
                    hc, hp = divmod(h, 2)
                    pb = hp * D_K
                    idn = ident[pb : pb + D_K, pb : pb + D_K]
                    for kc4 in range(KC // 4):
                        tp = rby_ps.tile([P, 4, D_K], f16, tag="rby",
                                         name=f"vt{h}{kc4}")
                        for j in range(4):
                            kc = kc4 * 4 + j
                            nc.tensor.transpose(
                                tp[:, j, :],
                                VT[pb : pb + D_K, hc,
                                   kc * P : (kc + 1) * P],
                                idn,
                            )
                        nc.vector.tensor_copy(
                            vp[:, kc4 * 4 : kc4 * 4 + 4, 0:D_K], tp[:]
                        )
                    vprime[h] = vp

            def emit_norm_late(ctx):
                # broadcast the reciprocal row across partitions with a K=1
                # fp16 matmul, then scale the unnormalized head outputs.
                for hc, qb_, xun, rec16 in ctx:
                    qs_ = slice(qb_ * QB, (qb_ + 1) * QB)
                    for hp in range(2):
                        rb_ps = rby_ps.tile([D_K, QB], f32, tag="rby",
                                            name=f"rb{hc}{hp}")
                        nc.tensor.matmul(
                            rb_ps[:],
                            ones16[0:1, :],
                            rec16[0:1, hp, :],
                            start=True,
                            stop=True,
                        )
                        if hp == 0:
                            nc.vector.tensor_tensor(
                                XT[0:D_K, hc, qs_], xun[:, 0, :], rb_ps[:],
                                mybir.AluOpType.mult,
                            )
                        else:
                            tmp = norm_pool.tile([D_K, QB], f16, tag="xtmp")
                            nc.vector.tensor_tensor(
                                tmp[:], xun[:, 1, :], rb_ps[:],
                                mybir.AluOpType.mult,
                            )
                            nc.sync.dma_start(XT[D_K:P, hc, qs_], tmp[:])

            def emit_outproj_chunk(qb_, sc4, ec, y_sb):
                # one ec-half of one 128-seq chunk: 2 matmuls + 1 copy
                sc = qb_ * 4 + sc4
                ps = rby_ps.tile([P, 512], f32, tag="rby",
                                 name=f"yp{sc4}{ec}")
                for dc in range(DSL // P):
                    nc.tensor.matmul(
                        ps[:],
                        XT[:, dc, sc * P : (sc + 1) * P],
                        woT_sb[:, dc, ec * 512 : (ec + 1) * 512],
                        start=(dc == 0),
                        stop=(dc == DSL // P - 1),
                    )
                nc.vector.tensor_copy(
                    y_sb[:, sc4, ec * 512 : (ec + 1) * 512], ps[:]
                )

            def emit_epilogue(hc, qb_, acc):
                # drain the pair's acc right away: one copy of both heads'
                # unnormalized outputs, one Ln + one Exp for both row-sum
                # reciprocals (1/sum = exp(-ln(sum)); the
                # natural_log_exp_and_others table serves both).
                xun = norm_pool.tile([D_K, 2, QB], f32, tag="xun",
                                     name=f"xun{hc}")
                nc.vector.tensor_copy(xun[:], acc[0:D_K, :, :])
                lnr = norm_pool.tile([1, 2, QB], f32, tag="lnr",
                                     name=f"ln{hc}", bufs=1)
                nc.scalar.activation(
                    lnr[:],
                    acc[D_K : D_K + 1, :, :],
                    mybir.ActivationFunctionType.Ln,
                )
                rec16 = norm_pool.tile([1, 2, QB], f16, tag="rec16",
                                       name=f"rc{hc}")
                nc.scalar.activation(
                    rec16[:],
                    lnr[:],
                    mybir.ActivationFunctionType.Exp,
                    scale=-1.0,
                )
                return [(hc, qb_, xun, rec16)]

            def emit_scores(heads, qb_, g, expm):
                # j-outer: the two heads' same-kc matmuls are adjacent and
                # live in disjoint PE row groups (bases 0/64).
                qs = slice(qb_ * QB, (qb_ + 1) * QB)
                hc = heads[0] // 2
                tiles = {}
                for h in heads:
                    tiles[h] = sc_ps.tile([P, 2, QB], f32, tag="sc",
                                          name=f"sc{h}{g}")
                for j in range(2):
                    kc = 2 * g + j
                    for h in heads:
                        hp = h & 1
                        pb = hp * D_K
                        nc.tensor.matmul(
                            tiles[h][:, j, :],
                            KT[pb : pb + D_K, hc, kc * P : (kc + 1) * P],
                            QT[pb : pb + D_K, hc, qs],
                            start=True,
                            stop=True,
                            tile_position=(pb, 0),
                        )
                for h in heads:
                    hp = h & 1
                    nc.scalar.activation(
                        expm[:, 2 * g : 2 * g + 2, hp, :],
                        tiles[h][:],
                        mybir.ActivationFunctionType.Exp,
                        scale=float(SCALE),
                    )

            def emit_attnv(heads, acc, expm, kcs):
                for kc in kcs:
                    for h in heads:
                        hp = h & 1
                        nc.tensor.matmul(
                            acc[0 : D_K + 1, hp, :],
                            vprime[h][:, kc, :],
                            expm[:, kc, hp, :],
                            start=(kc == 0),
                            stop=(kc == KC - 1),
                        )

            # ---- filler queue: dependency-free matmul work, popped into
            #      score slots in push order so the in-order PE queue always
            #      has eligible work while scores wait on the exp rotation ----
            from collections import deque
            fillq = deque()

            def pop_fill(budget):
                first = True
                while fillq and (first or fillq[0][0] <= budget):
                    cost, fn = fillq.popleft()
                    fn()
                    budget -= cost
                    first = False
                    if budget <= 0:
                        break

            def drain_fill():
                while fillq:
                    _, fn = fillq.popleft()
                    fn()

            # ---- startup: minimal prefix, then pair 0 with K/Q projection
            #      chunks as slot fillers ----
            emit_kproj(0, 0)
            emit_qproj(0, 0)
            for ns_, mc_ in ((1, 0), (2, 0), (3, 0), (0, 1), (1, 1),
                             (2, 1), (3, 1)):
                fillq.append((1700, lambda n=ns_, m=mc_: emit_kproj(n, m)))
            fillq.append((1700, lambda: emit_qproj(0, 1)))

            heads0 = (0, 1)
            heads1 = (2, 3)
            expm0 = exp_pool.tile([P, KC, 2, QB], f16, tag="exp", name="ex00")
            for g in range(NG):
                pop_fill(2000)
                emit_scores(heads0, 0, g, expm0)
            drain_fill()  # all K/Q work done before xk frees
            es_k.close()

            # xv staged only now (xk's 32KB freed); V DMA follows the k/q data
            xvpool = es_v.enter_context(tc.tile_pool(name="xv", bufs=1))
            for qt in range(4):
                xv_tiles.append(xvpool.tile([P, 2, S], f16, tag=f"xv{qt}",
                                            name=f"xv{qt}"))
            for ns in range(4):
                issue_xkv_ns("v", xv_tiles, ns)
            woT_sb = wpool.tile([P, DSL // P, HIDDEN], f16, tag="wo")
            nc.sync.dma_start(woT_sb[:], woT_d.rearrange("(c p) e -> p c e", p=P))

            # qb0 attention runs entirely through the queue: V projection,
            # V' builds, both pairs' attn@V, epilogues and norms.
            state = {"pn": None, "acc": None, "ysb": None, "epi": {}}
            accs = {}

            def drain_until_epi(pk_prev):
                # the single acc PSUM slot rotates pair->pair: the previous
                # pair's queued attn@V tail + epilogue must be emitted before
                # the next pair's in-slot attn@V allocates the slot
                while fillq and not state["epi"].get(pk_prev, False):
                    _, fn = fillq.popleft()
                    fn()

            def q_vproj(mc, ns):
                fillq.append((1700, lambda: emit_vproj(mc, ns)))

            def q_vprime(h):
                fillq.append((1800, lambda: emit_vprime((h,))))

            def q_attnv(pair_key, heads, expm, kc0):
                def fn():
                    if pair_key not in accs:
                        accs[pair_key] = acc_ps.tile(
                            [P, 2, QB], f32, tag="acc",
                            name=f"acc{pair_key[0]}{pair_key[1]}")
                    emit_attnv(heads, accs[pair_key], expm,
                               range(kc0, kc0 + 4))
                fillq.append((850, fn))

            def q_epi(pair_key, hc, qb_):
                def fn():
                    state["pn"] = emit_epilogue(hc, qb_, accs.pop(pair_key))
                    state["epi"][pair_key] = True
                fillq.append((300, fn))

            def q_norm():
                def fn():
                    if state["pn"] is not None:
                        emit_norm_late(state["pn"])
                        state["pn"] = None
                fillq.append((600, fn))

            def q_call(fn, cost=100):
                fillq.append((cost, fn))

            def q_outproj(qb_):
                def mk(c):
                    def fn():
                        if state["ysb"] is None:
                            state["ysb"] = ypool.tile(
                                [P, 4, HIDDEN], f16, tag="y",
                                name=f"y{qb_}")
                        emit_outproj_chunk(qb_, c // 2, c % 2, state["ysb"])
                        if c % 2 == 1:
                            nc.sync.dma_start(
                                y_r[:, qb_ * 4 + c // 2 :
                                    qb_ * 4 + c // 2 + 1, :],
                                state["ysb"][:, c // 2 : c // 2 + 1, :],
                            )
                        if c == 7:
                            state["ysb"] = None
                    return fn
                for c in range(8):
                    fillq.append((500, mk(c)))

            # push qb0's V chain + attention
            q_vproj(0, 0)
            q_vproj(0, 1)
            q_vproj(0, 2)
            q_vproj(0, 3)
            q_vprime(0)
            q_vprime(1)
            for kc0 in (0, 4, 8, 12):
                q_attnv((0, 0), heads0, expm0, kc0)
            q_epi((0, 0), 0, 0)
            q_vproj(1, 0)
            q_vproj(1, 1)
            q_norm()
            q_vproj(1, 2)
            q_vproj(1, 3)
            q_vprime(2)
            q_vprime(3)
            q_call(lambda: es_v.close())

            expm1 = exp_pool.tile([P, KC, 2, QB], f16, tag="exp", name="ex01")
            fillq.appendleft((1700, lambda: emit_qproj(1, 0)))
            for g in range(NG):
                pop_fill(3000)
                if g == 0:
                    emit_qproj(1, 1)
                emit_scores(heads1, 0, g, expm1)

            for kc0 in (0, 4, 8, 12):
                q_attnv((0, 1), heads1, expm1, kc0)
            q_epi((0, 1), 1, 0)
            q_call(lambda: issue_xq(2))
            q_norm()
            q_outproj(0)

            # ---- steady state: q-blocks 1..3, self attn@V in-slot (D=2),
            #      tails and epilogues through the queue ----
            D2 = 2
            prev_pk = (0, 1)
            for qb in range(1, N_QB):
                for hpair in range(2):
                    heads = (2 * hpair, 2 * hpair + 1)
                    pk = (qb, hpair)
                    drain_until_epi(prev_pk)
                    prev_pk = pk
                    expm = exp_pool.tile([P, KC, 2, QB], f16, tag="exp",
                                         name=f"ex{qb}{hpair}")
                    for g in range(NG):
                        if g >= D2:
                            if pk not in accs:
                                accs[pk] = acc_ps.tile(
                                    [P, 2, QB], f32, tag="acc",
                                    name=f"acc{qb}{hpair}")
                            emit_attnv(heads, accs[pk], expm,
                                       (2 * (g - D2), 2 * (g - D2) + 1))
                        pop_fill(700 if 2 <= g <= 7 else 1400)
                        emit_scores(heads, qb, g, expm)
                    q_attnv(pk, heads, expm, 12)
                    q_epi(pk, hpair, qb)
                    if qb < N_QB - 1:
                        fillq.append(
                            (1700, lambda q=qb, h=hpair: emit_qproj(q + 1, h)))
                        if hpair == 1 and qb + 2 < N_QB:
                            q_call(lambda q=qb: issue_xq(q + 2))
                    q_norm()
                    if hpair == 1:
                        q_outproj(qb)
            drain_fill()
    nc.finalize()
    return nc


_NC_CACHE = None


def _get_nc():
    global _NC_CACHE
    if _NC_CACHE is None:
        _NC_CACHE = _build_nc()
    return _NC_CACHE


def make_in_maps(q, k, v, Wq, bq, Wk, bk, Wv, bv, Wo):
    """Host-side sharding: per-core input dicts (core = b * G + g)."""
    f16 = np.float16
    qT = [np.ascontiguousarray(q[b].T).astype(f16) for b in range(B)]
    kT = [np.ascontiguousarray(k[b].T).astype(f16) for b in range(B)]
    vT = [np.ascontiguousarray(v[b].T).astype(f16) for b in range(B)]
    in_maps = []
    for core in range(B * G):
        b, g = divmod(core, G)
        sl = slice(g * DSL, (g + 1) * DSL)
        in_maps.append(
            {
                "qT": qT[b],
                "kT": kT[b],
                "vT": vT[b],
                "wqT": np.ascontiguousarray(Wq[sl, :].T).astype(f16),
                "wkT": np.ascontiguousarray(Wk[sl, :].T).astype(f16),
                "wvT": np.ascontiguousarray(Wv[sl, :].T).astype(f16),
                "woT": np.ascontiguousarray(Wo[:, sl].T).astype(f16),
                "bq": np.ascontiguousarray(bq[sl], np.float32),
                "bk": np.ascontiguousarray(bk[sl], np.float32),
                "bv": np.ascontiguousarray(bv[sl], np.float32),
            }
        )
    return in_maps


def kernel(q, k, v, Wq, bq, Wk, bk, Wv, bv, Wo, bo):
    from concourse.bass_utils import run_bass_kernel_spmd

    q, k, v = (np.asarray(a, np.float32) for a in (q, k, v))
    Wq, Wk, Wv, Wo = (np.asarray(a, np.float32) for a in (Wq, Wk, Wv, Wo))
    bq, bk, bv, bo = (np.asarray(a, np.float32) for a in (bq, bk, bv, bo))

    nc = _get_nc()
    in_maps = make_in_maps(q, k, v, Wq, bq, Wk, bk, Wv, bv, Wo)
    res = run_bass_kernel_spmd(nc, in_maps, core_ids=list(range(B * G)))

    out = np.zeros((B, S, HIDDEN), np.float32)
    for b in range(B):
        acc = np.zeros((S, HIDDEN), np.float32)
        for g in range(G):
            acc += res.results[b * G + g]["y"].astype(np.float32)
        out[b] = acc + bo
    return out
